# revision 40
# baseline (speedup 1.0000x reference)
"""AFNO2D Trainium kernel: block-parallel over 8 NeuronCores (one 96-ch block each).

Per core, per batch b (4 sequential):
  S1 FFT-H  (DW): per c:  psum[w,130]  = X[:,:,c].T @ [BHr|BHi] (g=0..64 only;
              g>=65 reconstructed in S2 via Hermitian symmetry of the real input)
  S2 FFT-W  (DW): per g:  psum[c,130]  = T1r[:,:,mg].T@AW +- T1i[:,:,mg].T@AWn
  S3 MLP1   (MW): chunks: psum[c,n]    = w1A.T@Sr + w1B.T@Si (bias via ones-row) -> relu -> o1r/o1i
  S4 MLP2   (DW): per f:  psum[g,192]  = o1r[:,:,f].T@W2A + o1i[:,:,f].T@W2B -> softshrink -> yr/yi [g,f,c]
  S5 iFFT-H (DW): per c:  psum[f,256]  = yr[:,:,c].T@CH + yi[:,:,c].T@CHn -> Z [f,c,(h r|i)]
  S6 iFFT-W (DW): per c:  psum[h,w]    = Zr.T@Dr + Zi.T@Di ; out = psum + x  -> staging -> HBM

r/i pairs live interleaved in one tile (T1/S/Z) so each PSUM evacuation is a
single big op, alternating DVE/ACT. All matmuls bf16, psum f32. Input loaded
with DMA f32->bf16 cast (SWDGE); output staged bf16, DMA-cast to f32 on store.
"""
import numpy as np
import ml_dtypes

import concourse.bass as bass
import concourse.mybir as mybir
import concourse.tile as tile
from concourse import bacc
from concourse.bass_utils import run_bass_kernel_spmd

BF16 = mybir.dt.bfloat16
F32 = mybir.dt.float32
N = 128          # H = W = 128
WF = 65          # rfft bins along W
C = 96           # channels per block (per core)
B = 4
LAM = 0.01
NCORES = 8
NPOS = N * WF    # 8320 positions (g-major: pos = g*65 + f)


# ---------------------------------------------------------------- host consts
def _make_consts():
    inv = 1.0 / np.sqrt(N)
    k = np.arange(N)
    f = np.arange(WF)
    hg = np.outer(k, k) * (2 * np.pi / N)
    wf = np.outer(k, f) * (2 * np.pi / N)
    BHr = np.cos(hg) * inv
    BHi = -np.sin(hg) * inv
    AWr = np.cos(wf) * inv
    AWi = -np.sin(wf) * inv
    CHr = np.cos(hg) * inv
    CHi = np.sin(hg) * inv
    mult = np.where((f == 0) | (f == WF - 1), 1.0, 2.0)
    fw = np.outer(f, k) * (2 * np.pi / N)
    Dr = mult[:, None] * np.cos(fw) * inv
    Di = -mult[:, None] * np.sin(fw) * inv
    bf = ml_dtypes.bfloat16
    return {
        # forward FFT-H, kept half-spectrum g = 0..64
        "bh": np.concatenate([BHr[:, :WF], BHi[:, :WF]], 1).astype(bf),  # [128, 130]
        "aw": np.concatenate([AWr, AWi], 1).astype(bf),      # [128, 130]
        "awn": np.concatenate([-AWi, AWr], 1).astype(bf),    # [128, 130]
        "awn2": np.concatenate([AWi, -AWr], 1).astype(bf),   # [128, 130] mirror-g variant
        "ch": np.concatenate([CHr, CHi], 1).astype(bf),      # [128, 256]
        "chn": np.concatenate([-CHi, CHr], 1).astype(bf),    # [128, 256]
        "dr": Dr.astype(bf),                                 # [65, 128]
        "di": Di.astype(bf),                                 # [65, 128]
        "ones": np.ones((1, 2 * NPOS), bf),
    }


def _make_weights(w1k, b1k, w2k, b2k):
    """w1k: [2, 96, 96] f32 for this core's block; returns augmented bf16 mats."""
    bf = ml_dtypes.bfloat16
    z = np.zeros((1, C), np.float32)
    w1a = np.concatenate([w1k[0], b1k[0][None]], 0).astype(bf)    # [97, 96]
    w1b = np.concatenate([-w1k[1], z], 0).astype(bf)
    w1c = np.concatenate([w1k[1], b1k[1][None]], 0).astype(bf)
    w1d = np.concatenate([w1k[0], z], 0).astype(bf)
    w2a = np.concatenate(
        [np.concatenate([w2k[0], w2k[1]], 1),
         np.concatenate([b2k[0], b2k[1]])[None]], 0).astype(bf)   # [97, 192]
    w2b = np.concatenate(
        [np.concatenate([-w2k[1], w2k[0]], 1), np.zeros((1, 2 * C), np.float32)], 0
    ).astype(bf)
    return {"w1a": w1a, "w1b": w1b, "w1c": w1c, "w1d": w1d, "w2a": w2a, "w2b": w2b}


# ---------------------------------------------------------------- bass build
def build_nc():
    nc = bacc.Bacc()

    x_d = nc.declare_dram_parameter("x", [B, N, N, C], BF16, isOutput=False)
    out_d = nc.declare_dram_parameter("out", [B, N, N, C], BF16, isOutput=True)
    cdecl = {}
    for name, shape in [
        ("bh", [N, 2 * WF]), ("aw", [N, 2 * WF]), ("awn", [N, 2 * WF]),
        ("awn2", [N, 2 * WF]),
        ("ch", [N, 2 * N]), ("chn", [N, 2 * N]), ("dr", [WF, N]), ("di", [WF, N]),
        ("w1a", [C + 1, C]), ("w1b", [C + 1, C]), ("w1c", [C + 1, C]),
        ("w1d", [C + 1, C]), ("w2a", [C + 1, 2 * C]), ("w2b", [C + 1, 2 * C]),
        ("ones", [1, 2 * NPOS]),
    ]:
        cdecl[name] = nc.declare_dram_parameter(name, shape, BF16, isOutput=False)

    Relu = mybir.ActivationFunctionType.Relu
    Copy = mybir.ActivationFunctionType.Copy

    def evac(which, out_ap, in_ap):
        """Alternate PSUM evacuation copies between DVE and ACT."""
        if which % 2 == 0:
            nc.vector.tensor_copy(out_ap, in_ap)
        else:
            nc.scalar.activation(out_ap, in_ap, Copy)

    from contextlib import ExitStack
    with tile.TileContext(nc, pool_alloc_mode="queue") as tc, ExitStack() as ctx:
        consts = ctx.enter_context(tc.tile_pool(name="consts", bufs=1))
        persist = ctx.enter_context(tc.tile_pool(name="persist", bufs=1))
        xpool = ctx.enter_context(tc.tile_pool(name="xin", bufs=2))
        stage = ctx.enter_context(tc.tile_pool(name="stage", bufs=1))
        outp = ctx.enter_context(tc.tile_pool(name="outp", bufs=1))
        zpool = ctx.enter_context(tc.tile_pool(name="zp", bufs=2))
        zp1 = ctx.enter_context(tc.tile_pool(name="zp1", bufs=1))
        psA = ctx.enter_context(tc.tile_pool(name="psA", bufs=3, space="PSUM"))
        psB = ctx.enter_context(tc.tile_pool(name="psB", bufs=2, space="PSUM"))

        # constants into SBUF
        cs = {}
        for name, t in cdecl.items():
            if name == "ones":
                continue   # DMA'd straight into the persist tiles' bias row
            sb = consts.tile(list(t.shape), BF16, tag=name)
            nc.sync.dma_start(out=sb, in_=t[:, :])
            cs[name] = sb

        # persistent intermediates; ones-row (bias trick) DMA'd once
        S = persist.tile([C + 1, N, 2 * WF], BF16, tag="S")      # [c, g, (r65|i65)]
        o1r = persist.tile([C + 1, WF, N], BF16, tag="o1r")      # [c, f, g]
        o1i = persist.tile([C + 1, WF, N], BF16, tag="o1i")
        nc.sync.dma_start(out=S[C : C + 1, :, :], in_=cdecl["ones"][:, : 2 * NPOS])
        nc.sync.dma_start(out=o1r[C : C + 1, :, :], in_=cdecl["ones"][:, :NPOS])
        nc.sync.dma_start(out=o1i[C : C + 1, :, :], in_=cdecl["ones"][:, :NPOS])

        for b in range(B):
            # ---- load input (host pre-cast to bf16; SWDGE, no cast)
            Xb = xpool.tile([N, N, C], BF16, tag="xb")   # [h, w, c]
            nc.gpsimd.dma_start(out=Xb, in_=x_d[b, :, :, :])

            # [w, (r65|i65), c-pad128]: c contiguous so S2's LDWEIGHTS gets FWL
            T1 = stage.tile([N, 2 * WF, N], BF16, tag="t1")

            # ---- S1: FFT over H, half spectrum (DW, per c); 6 c per psum tile
            # (3 130-wide outputs per 2KB bank: offsets 0/130/260 within a bank)
            for i, c0 in enumerate(range(0, C, 6)):
                ps = psA.tile([N, 1024], F32, tag="psA")
                for ci in range(6):
                    off = (ci // 3) * 512 + (ci % 3) * 130
                    nc.tensor.matmul(
                        ps[:, off : off + 2 * WF],
                        lhsT=Xb[:, :, c0 + ci], rhs=cs["bh"], start=True, stop=True)
                psv = ps.rearrange("w (bk s) -> w bk s", bk=2)[:, :, 0 : 3 * 2 * WF]
                psv = psv.rearrange("w bk (k s) -> w bk k s", k=3)
                dst = T1[:, :, c0 : c0 + 6].rearrange("w s (bk k) -> w bk k s", bk=2)
                evac(i, dst, psv)

            # ---- S2: FFT over W (DW, per g); 6 g per psum tile (3 per bank).
            # g >= 65: T1(g) = conj(T1(128-g)) -> same lhsT, negated rhs for T1i.
            # full 128-col lhsT (c padded) -> FWL; rows 96.. of psum are junk
            for i, g0 in enumerate(range(0, N, 6)):
                ng = min(6, N - g0)
                ps = psA.tile([N, 1024], F32, tag="psA")
                for gi in range(ng):
                    g = g0 + gi
                    mg = g if g < WF else N - g
                    rhs2 = cs["awn"] if g < WF else cs["awn2"]
                    off = (gi // 3) * 512 + (gi % 3) * 130
                    nc.tensor.matmul(
                        ps[:, off : off + 2 * WF],
                        lhsT=T1[:, mg, :], rhs=cs["aw"], start=True, stop=False)
                    nc.tensor.matmul(
                        ps[:, off : off + 2 * WF],
                        lhsT=T1[:, WF + mg, :], rhs=rhs2, start=False, stop=True)
                if ng == 6:
                    psv = ps.rearrange("c (bk s) -> c bk s", bk=2)[0:C, :, 0 : 3 * 2 * WF]
                    psv = psv.rearrange("c bk (k s) -> c bk k s", k=3)
                    dst = S[0:C, g0 : g0 + 6, :].rearrange("c (bk k) s -> c bk k s", bk=2)
                    evac(i, dst, psv)
                else:   # tail group of 2 (bank 0, slots 0/1)
                    psv = ps[0:C, 0 : 2 * 130].rearrange("c (k s) -> c k s", k=2)
                    evac(i, S[0:C, g0 : g0 + 2, :], psv)

            # ---- S3: MLP layer 1 (MW over 7-g chunks = 455 positions)
            o1rv = o1r.rearrange("c f g -> c g f")
            o1iv = o1i.rearrange("c f g -> c g f")
            GC = 7
            for g0 in range(0, N, GC):
                ng = min(GC, N - g0)
                n = ng * WF
                Srv = S[:, g0 : g0 + ng, 0:WF]
                Siv = S[:, g0 : g0 + ng, WF : 2 * WF]
                pr = psA.tile([C, 512], F32, tag="psA")
                pi = psB.tile([C, 512], F32, tag="psB")
                nc.tensor.matmul(pr[:, :n], lhsT=cs["w1a"], rhs=Srv,
                                 start=True, stop=False)
                nc.tensor.matmul(pr[:, :n], lhsT=cs["w1b"], rhs=Siv,
                                 start=False, stop=True)
                nc.tensor.matmul(pi[:, :n], lhsT=cs["w1c"], rhs=Srv,
                                 start=True, stop=False)
                nc.tensor.matmul(pi[:, :n], lhsT=cs["w1d"], rhs=Siv,
                                 start=False, stop=True)
                prv = pr[:, :n].rearrange("c (g f) -> c g f", g=ng)
                piv = pi[:, :n].rearrange("c (g f) -> c g f", g=ng)
                nc.scalar.activation(o1rv[0:C, g0 : g0 + ng, :], prv, Relu)
                nc.vector.tensor_scalar_max(o1iv[0:C, g0 : g0 + ng, :], piv, 0.0)

            # ---- S4: MLP layer 2 + softshrink (DW, per f); batch 4 f per psum
            yr = stage.tile([N, WF, C], BF16, tag="yr")   # [g, f, c]
            yi = stage.tile([N, WF, C], BF16, tag="yi")
            for f0 in range(0, WF, 4):
                nf = min(4, WF - f0)
                ps = psA.tile([N, 4 * 256], F32, tag="psA")
                for fi in range(nf):
                    f = f0 + fi
                    nc.tensor.matmul(
                        ps[:, fi * 256 : fi * 256 + 192],
                        lhsT=o1r[:, f, :], rhs=cs["w2a"], start=True, stop=False)
                    nc.tensor.matmul(
                        ps[:, fi * 256 : fi * 256 + 192],
                        lhsT=o1i[:, f, :], rhs=cs["w2b"], start=False, stop=True)
                # softshrink: u = copy(psum); t = clamp(u); y = u - t
                u = zpool.tile([N, 4, 192], BF16, tag="u")
                t = zp1.tile([N, 4, 192], BF16, tag="t")
                psv = ps.rearrange("g (f s) -> g f s", f=4)
                nc.scalar.activation(u[:, 0:nf, :], psv[:, 0:nf, 0:192], Copy)
                nc.vector.tensor_scalar(
                    t.rearrange("g f s -> g (f s)")[:, : nf * 192],
                    u.rearrange("g f s -> g (f s)")[:, : nf * 192], LAM, -LAM,
                    mybir.AluOpType.min, mybir.AluOpType.max)
                nc.vector.tensor_tensor(
                    yr[:, f0 : f0 + nf, :], u[:, 0:nf, 0:C], t[:, 0:nf, 0:C],
                    mybir.AluOpType.subtract)
                nc.gpsimd.tensor_tensor(
                    yi[:, f0 : f0 + nf, :], u[:, 0:nf, C : 2 * C],
                    t[:, 0:nf, C : 2 * C], mybir.AluOpType.subtract)

            # ---- S5 + S6 per 4c (pipelined): iFFT-H then iFFT-W + residual
            ob = outp.tile([N, N, C], BF16, tag="ob")     # [h, w, c] output staging
            obv = ob.rearrange("h w c -> h c w")
            Xbv = Xb.rearrange("h w c -> h c w")
            for i, c0 in enumerate(range(0, C, 4)):
                Z = zpool.tile([WF, 4, 2 * N], BF16, tag="z")   # [f, c, (h_r|h_i)]
                ps5 = psA.tile([WF, 4 * 256], F32, tag="psA")
                for ci in range(4):
                    c = c0 + ci
                    nc.tensor.matmul(
                        ps5[:, ci * 256 : ci * 256 + 2 * N],
                        lhsT=yr[:, :, c], rhs=cs["ch"], start=True, stop=False)
                    nc.tensor.matmul(
                        ps5[:, ci * 256 : ci * 256 + 2 * N],
                        lhsT=yi[:, :, c], rhs=cs["chn"], start=False, stop=True)
                if i % 4 == 0:
                    nc.vector.tensor_copy(Z, ps5.rearrange("f (c s) -> f c s", c=4))
                else:
                    nc.scalar.activation(Z, ps5.rearrange("f (c s) -> f c s", c=4), Copy)
                ps6 = psB.tile([N, 4 * N], F32, tag="psB")
                for ci in range(4):
                    nc.tensor.matmul(ps6[:, ci * N : (ci + 1) * N],
                                     lhsT=Z[:, ci, 0:N], rhs=cs["dr"],
                                     start=True, stop=False)
                    nc.tensor.matmul(ps6[:, ci * N : (ci + 1) * N],
                                     lhsT=Z[:, ci, N : 2 * N], rhs=cs["di"],
                                     start=False, stop=True)
                p6 = ps6.rearrange("h (c w) -> h c w", c=4)
                nc.vector.tensor_tensor(
                    obv[:, c0 : c0 + 4, :], p6, Xbv[:, c0 : c0 + 4, :],
                    mybir.AluOpType.add)

            # ---- store (bf16; host casts to f32 after gather)
            nc.gpsimd.dma_start(out=out_d[b, :, :, :], in_=ob)

    if not nc.is_finalized():
        nc.finalize()
    return nc


_NC_CACHE = None


def _get_nc():
    global _NC_CACHE
    if _NC_CACHE is None:
        _NC_CACHE = build_nc()
    return _NC_CACHE


def kernel(x, w1, b1, w2, b2):
    x = np.ascontiguousarray(np.asarray(x, dtype=np.float32))
    consts = _make_consts()
    in_maps = []
    for k in range(NCORES):
        sl = slice(k * C, (k + 1) * C)
        m = {"x": np.ascontiguousarray(x[:, :, :, sl]).astype(ml_dtypes.bfloat16)}
        m.update(consts)
        m.update(_make_weights(
            np.asarray(w1, np.float32)[:, k], np.asarray(b1, np.float32)[:, k],
            np.asarray(w2, np.float32)[:, k], np.asarray(b2, np.float32)[:, k]))
        in_maps.append(m)
    global _last_in_maps
    _last_in_maps = in_maps
    nc = _get_nc()
    res = run_bass_kernel_spmd(nc, in_maps, list(range(NCORES)))
    out = np.concatenate([r["out"] for r in res.results], axis=-1)
    return out.astype(np.float32)


_last_in_maps = None
